# revision 1
# baseline (speedup 1.0000x reference)
"""Trainium2 Bass kernel for nn_Head_75118978007668.

Computes, for x:[B,S,D], concept_map(cm):[D,D,D] (B=4, S=2048, D=128):
    s[b,t] = sum_{j<t} lam^(t-j) x[b,j]          (lam = 1/1.2 decayed prefix sum)
    out[b,t,f] = sum_{d,e} x[b,t,d] * s[b,t,e] * cm[f,d,e]

Sharding: 8 cores, each owns 1024 contiguous positions of one batch row
(4 rows x 2 halves).  The scan carry across the half-split is recovered
exactly (to fp32) from a 256-position halo, since lam^256 ~ 4.5e-21 is far
below fp32 resolution.

Per-core dataflow (positions tiled 8 x 128):
  - carries: small PE matmuls build s(tile_start) for all 8 tiles at once
  - s tiles: triangular matmul  s = L @ x_tile + pow (x) carry   (PE, fp32)
  - main:    Y[p, (e,f)] = xT_tile.T @ W2   (PE, fp32r, N=512 chunks)
             acc[p,f]  += s[p,e] * Y[p,(e,f)]   (DVE scalar_tensor_tensor)
  where W2[d, e*128+f] = cm[f, d, e]  (host-transposed).
"""

import numpy as np

import concourse.bass as bass
import concourse.tile as tile
from concourse import bacc, mybir
from concourse.bass import ds, ts
from concourse.bass_utils import run_bass_kernel_spmd

B, S, D = 4, 2048, 128
NCORES = 8
CHUNK = S // 2          # positions per core (1024)
NT = CHUNK // 128       # position tiles per core (8)
P = 128
HALO = 256
F32 = mybir.dt.float32
F32R = mybir.dt.float32r

# match the reference's fp32 constant 1.2 exactly
LAM = 1.0 / np.float64(np.float32(1.2))

MAIN_MM_DTYPE = F32R    # flip to F32 if fp32r hw numerics are too loose

_CACHE = {}
LAST_RESULTS = None


def _host_constants():
    k = np.arange(P, dtype=np.float64)
    i = k
    # LT[i, k] = L[k, i] = lam^(k-i) for i < k   (lhsT of the triangular scan)
    LT = np.where(i[:, None] < k[None, :], LAM ** (k[None, :] - i[:, None]), 0.0)
    powv = (LAM ** k)[None, :]                      # [1, 128]
    vw = (LAM ** (P - i))[:, None]                  # [128, 1]
    j = np.arange(HALO, dtype=np.float64)           # halo weights lam^(256-j)
    hw = (LAM ** (HALO - j)).reshape(2, P).T        # [128, 2]  hw[i, u] = lam^(256-(u*128+i))
    # M9[t, jj]: c_t = sum_jj M9[t, jj] * V9[jj];  V9 = [c0, v_0..v_7]
    t = np.arange(NT, dtype=np.float64)
    M9 = np.zeros((NT, NT + 1), dtype=np.float64)
    M9[:, 0] = LAM ** (P * t)
    for tt in range(NT):
        for jj in range(tt):
            M9[tt, jj + 1] = LAM ** (P * (tt - 1 - jj))
    LT9 = M9.T                                      # [9, 8]
    f32 = np.float32
    return {
        "lt": LT.astype(f32),
        "powv": powv.astype(f32),
        "vw": vw.astype(f32),
        "hw": hw.astype(f32),
        "lt9": LT9.astype(f32),
    }


def _build_nc():
    nc = bacc.Bacc("TRN2", target_bir_lowering=False, debug=False,
                   num_devices=NCORES)
    x_d = nc.declare_dram_parameter("x", [P, NT, P], F32, isOutput=False)        # [i, t, e]
    xt_d = nc.declare_dram_parameter("xt", [P, CHUNK], MAIN_MM_DTYPE, isOutput=False)  # [d, p]
    halo_d = nc.declare_dram_parameter("halo", [P, 2, P], F32, isOutput=False)   # [i, u, e]
    w2_d = nc.declare_dram_parameter("w2", [P, P * P], MAIN_MM_DTYPE, isOutput=False)  # [d, (e,f)]
    lt_d = nc.declare_dram_parameter("lt", [P, P], F32, isOutput=False)
    pow_d = nc.declare_dram_parameter("powv", [1, P], F32, isOutput=False)
    vw_d = nc.declare_dram_parameter("vw", [P, 1], F32, isOutput=False)
    hw_d = nc.declare_dram_parameter("hw", [P, 2], F32, isOutput=False)
    lt9_d = nc.declare_dram_parameter("lt9", [NT + 1, NT], F32, isOutput=False)
    out_d = nc.declare_dram_parameter("out", [P, NT, P], F32, isOutput=True)  # [p, t, f]

    mult = mybir.AluOpType.mult
    add = mybir.AluOpType.add

    with tile.TileContext(nc) as tc:
        with tc.tile_pool(name="consts", bufs=1) as consts:
            w2_sb = [consts.tile([P, 2048], MAIN_MM_DTYPE, name=f"w2_sb{i}")
                     for i in range(8)]
            xt_sb = consts.tile([P, CHUNK], MAIN_MM_DTYPE)
            x_sb = consts.tile([P, NT, P], F32)
            halo_sb = consts.tile([P, 2, P], F32)
            lt_sb = consts.tile([P, P], F32)
            pow_sb = consts.tile([1, P], F32)
            vw_sb = consts.tile([P, 1], F32)
            hw_sb = consts.tile([P, 2], F32)
            lt9_sb = consts.tile([NT + 1, NT], F32)
            v9_sb = consts.tile([NT + 1, P], F32)
            c0_sb = consts.tile([1, P], F32)
            va_sb = consts.tile([1, 4 * P], F32)
            vb_sb = consts.tile([1, 4 * P], F32)
            c8_sb = consts.tile([NT, P], F32)
            c_all = consts.tile([1, NT * P], F32)    # [1, (t,e)] carries
            s_sb = consts.tile([P, NT, P], F32)      # [p, t, e]
            acc = consts.tile([P, NT, P], F32)       # [p, t, f]

            for i in range(8):
                nc.sync.dma_start(out=w2_sb[i][:, :],
                                  in_=w2_d[:, ds(2048 * i, 2048)])
            nc.sync.dma_start(out=xt_sb[:, :], in_=xt_d[:, :])
            nc.sync.dma_start(out=x_sb[:, :, :], in_=x_d[:, :, :])
            nc.sync.dma_start(out=halo_sb[:, :, :], in_=halo_d[:, :, :])
            nc.sync.dma_start(out=lt_sb[:, :], in_=lt_d[:, :])
            nc.sync.dma_start(out=pow_sb[:, :], in_=pow_d[:, :])
            nc.sync.dma_start(out=vw_sb[:, :], in_=vw_d[:, :])
            nc.sync.dma_start(out=hw_sb[:, :], in_=hw_d[:, :])
            nc.sync.dma_start(out=lt9_sb[:, :], in_=lt9_d[:, :])

            nc.vector.memset(acc[:, :, :], 0.0)

            # ---- carries: c_t = s[tile_start t] for all 8 tiles ----
            with tc.tile_pool(name="psum_c", bufs=1, space="PSUM") as psum_c:
                c0_ps = psum_c.tile([1, P], F32)
                nc.tensor.matmul(c0_ps[:, :], lhsT=hw_sb[:, 0:1],
                                 rhs=halo_sb[:, 0, :], start=True, stop=False)
                nc.tensor.matmul(c0_ps[:, :], lhsT=hw_sb[:, 1:2],
                                 rhs=halo_sb[:, 1, :], start=False, stop=True)
                vps_a = psum_c.tile([1, 4 * P], F32, tag="vps_a")
                vps_b = psum_c.tile([1, 4 * P], F32, tag="vps_b")
                nc.tensor.matmul(vps_a[:, :], lhsT=vw_sb[:, :],
                                 rhs=x_sb[:, 0:4, :], start=True, stop=True)
                nc.tensor.matmul(vps_b[:, :], lhsT=vw_sb[:, :],
                                 rhs=x_sb[:, 4:8, :], start=True, stop=True)
                nc.vector.tensor_copy(c0_sb[:, :], c0_ps[:, :])
                nc.vector.tensor_copy(va_sb[:, :], vps_a[:, :])
                nc.vector.tensor_copy(vb_sb[:, :], vps_b[:, :])
                nc.sync.dma_start(out=v9_sb[0:1, :], in_=c0_sb[:, :])
                nc.sync.dma_start(out=v9_sb[1:5, :], in_=va_sb[:, :])
                nc.sync.dma_start(out=v9_sb[5:9, :], in_=vb_sb[:, :])
                c_ps = psum_c.tile([NT, P], F32, tag="c_ps")
                nc.tensor.matmul(c_ps[:, :], lhsT=lt9_sb[:, :],
                                 rhs=v9_sb[:, :], start=True, stop=True)
                nc.vector.tensor_copy(c8_sb[:, :], c_ps[:, :])
                nc.sync.dma_start(out=c_all[:, :], in_=c8_sb[:, :])

            # ---- s tiles: s = L @ x_t + pow (x) c_t ----
            with tc.tile_pool(name="psum_s", bufs=2, space="PSUM") as psum_s:
                for t in range(NT):
                    sp = psum_s.tile([P, P], F32)
                    nc.tensor.matmul(sp[:, :], lhsT=lt_sb[:, :],
                                     rhs=x_sb[:, t, :], start=True, stop=False)
                    nc.tensor.matmul(sp[:, :], lhsT=pow_sb[:, :],
                                     rhs=c_all[:, ts(t, P)], start=False, stop=True)
                    nc.vector.tensor_copy(s_sb[:, t, :], sp[:, :])

            # ---- main: Y = xT_t.T @ W2 chunks; acc += s_e * Y_e ----
            with tc.tile_pool(name="psum_y", bufs=8, space="PSUM") as psum_y:
                for t in range(NT):
                    xt_t = xt_sb[:, ts(t, P)]
                    for c in range(32):
                        yp = psum_y.tile([P, 512], F32)
                        nc.tensor.matmul(
                            yp[:, :], lhsT=xt_t,
                            rhs=w2_sb[c // 4][:, ds(512 * (c % 4), 512)],
                            start=True, stop=True)
                        for jj in range(4):
                            e = 4 * c + jj
                            nc.vector.scalar_tensor_tensor(
                                out=acc[:, t, :],
                                in0=yp[:, ts(jj, P)],
                                scalar=s_sb[:, t, e:e + 1],
                                in1=acc[:, t, :],
                                op0=mult, op1=add)

            nc.sync.dma_start(out=out_d[:, :, :], in_=acc[:, :, :])
    nc.finalize()
    return nc


def _get_nc():
    if "nc" not in _CACHE:
        _CACHE["nc"] = _build_nc()
    return _CACHE["nc"]


def kernel(x, concept_map, _trace=False):
    global LAST_RESULTS
    x = np.asarray(x, dtype=np.float32)
    cm = np.asarray(concept_map, dtype=np.float32)
    assert x.shape == (B, S, D) and cm.shape == (D, D, D)

    consts = _host_constants()
    # W2[d, e*128+f] = cm[f, d, e]
    w2 = np.ascontiguousarray(np.transpose(cm, (1, 2, 0)).reshape(D, D * D))

    in_maps = []
    for core in range(NCORES):
        b, half = divmod(core, 2)
        lo = half * CHUNK
        xc = x[b, lo:lo + CHUNK]                          # [1024, 128]
        # [i, t, e] interleaved layout (partition = within-tile position)
        x_il = np.ascontiguousarray(
            xc.reshape(NT, P, D).transpose(1, 0, 2))
        xt = np.ascontiguousarray(xc.T)                   # [d, p]
        if half == 0:
            halo = np.zeros((P, 2, D), dtype=np.float32)
        else:
            h = x[b, lo - HALO:lo]                        # [256, 128]
            halo = np.ascontiguousarray(h.reshape(2, P, D).transpose(1, 0, 2))
        in_maps.append({
            "x": x_il, "xt": xt, "halo": halo, "w2": w2, **consts,
        })

    nc = _get_nc()
    res = run_bass_kernel_spmd(nc, in_maps, list(range(NCORES)), trace=_trace)
    LAST_RESULTS = res

    out = np.empty((B, S, D), dtype=np.float32)
    for core in range(NCORES):
        b, half = divmod(core, 2)
        o = res.results[core]["out"]                      # [p, t, f]
        out[b, half * CHUNK:(half + 1) * CHUNK] = (
            o.transpose(1, 0, 2).reshape(CHUNK, D))
    return out



# revision 5
# speedup vs baseline: 1.2942x; 1.2942x over previous
"""Trainium2 Bass kernel for nn_Head_75118978007668.

Computes, for x:[B,S,D], concept_map(cm):[D,D,D] (B=4, S=2048, D=128):
    s[b,t] = sum_{j<t} lam^(t-j) x[b,j]          (lam = 1/1.2 decayed prefix sum)
    out[b,t,f] = sum_{d,e} x[b,t,d] * s[b,t,e] * cm[f,d,e]

Sharding: 8 cores, each owns 1024 contiguous positions of one batch row
(4 rows x 2 halves).  The scan carry across the half-split is recovered
exactly (to fp32) from a 256-position halo, since lam^256 ~ 4.5e-21 is far
below fp32 resolution.

Per-core dataflow (positions tiled 8 x 128):
  - carries: small PE matmuls build s(tile_start) for all 8 tiles at once
  - s tiles: triangular matmul  s = L @ x_tile + pow (x) carry   (PE, fp32)
  - main:    Y[p, (e,f)] = xT_tile.T @ W2   (PE, bf16, N=512 chunks)
  - contraction acc[p,f] += s[p,e] * Y[p,(e,f)] split across 3 engine paths:
      A: DVE scalar_tensor_tensor direct from PSUM (fp32)
      D: ACT 512-wide scaled? no - plain copy PSUM->SBUF bf16, then DVE STT
      E: same ACT copy feeding GPSIMD STT (own accumulator)
  where W2[d, e*128+f] = cm[f, d, e]  (host-transposed).
"""

import numpy as np
import ml_dtypes

import concourse.bass as bass
import concourse.tile as tile
from concourse import bacc, mybir
from concourse.bass import ds, ts
from concourse.bass_utils import run_bass_kernel_spmd

B, S, D = 4, 2048, 128
NCORES = 8
CHUNK = S // 2          # positions per core (1024)
NT = CHUNK // 128       # position tiles per core (8)
P = 128
HALO = 256
F32 = mybir.dt.float32
BF16 = mybir.dt.bfloat16

# match the reference's fp32 constant 1.2 exactly
LAM = 1.0 / np.float64(np.float32(1.2))

# contraction routing: per 4-e chunk (32 chunks/tile), one of three paths:
#  "direct": 4 DVE STTs straight from PSUM                  (DVE 303ns/e)
#  "copied": ACT 512-wide bf16 copy -> 4 DVE STTs from SBUF (ACT 161 + DVE 238)
#  "gp":     4 ACT per-e scaled copies -> 4 GPSIMD TT-adds  (ACT 324 + GP ~400)
# per-tile chunk counts (sum = 32); pattern interleaved below
N_DIRECT = 13
N_COPIED = 6
N_GP = 13

_CACHE = {}
LAST_RESULTS = None


def _host_constants():
    k = np.arange(P, dtype=np.float64)
    i = k
    # LT[i, k] = L[k, i] = lam^(k-i) for i < k   (lhsT of the triangular scan)
    LT = np.where(i[:, None] < k[None, :], LAM ** (k[None, :] - i[:, None]), 0.0)
    powv = (LAM ** k)[None, :]                      # [1, 128]
    vw = (LAM ** (P - i))[:, None]                  # [128, 1]
    j = np.arange(HALO, dtype=np.float64)           # halo weights lam^(256-j)
    hw = (LAM ** (HALO - j)).reshape(2, P).T        # [128, 2]  hw[i, u] = lam^(256-(u*128+i))
    # M9[t, jj]: c_t = sum_jj M9[t, jj] * V9[jj];  V9 = [c0, v_0..v_7]
    t = np.arange(NT, dtype=np.float64)
    M9 = np.zeros((NT, NT + 1), dtype=np.float64)
    M9[:, 0] = LAM ** (P * t)
    for tt in range(NT):
        for jj in range(tt):
            M9[tt, jj + 1] = LAM ** (P * (tt - 1 - jj))
    LT9 = M9.T                                      # [9, 8]
    f32 = np.float32
    return {
        "lt": LT.astype(f32),
        "powv": powv.astype(f32),
        "vw": vw.astype(f32),
        "hw": hw.astype(f32),
        "lt9": LT9.astype(f32),
    }


def _chunk_routes():
    """routes[t][c] in {'direct','copied','gp'}, interleaved for overlap."""
    base = []
    # build one tile's 32-chunk pattern by largest-remainder interleave
    counts = {"direct": N_DIRECT, "copied": N_COPIED, "gp": N_GP}
    assert sum(counts.values()) == 32
    acc = {k: 0.0 for k in counts}
    for c in range(32):
        for k in counts:
            acc[k] += counts[k] / 32.0
        pick = max(acc, key=lambda k: acc[k])
        acc[pick] -= 1.0
        base.append(pick)
    return [list(base) for _ in range(NT)]


def _build_nc():
    nc = bacc.Bacc("TRN2", target_bir_lowering=False, debug=False,
                   num_devices=NCORES)
    x_d = nc.declare_dram_parameter("x", [P, NT, P], F32, isOutput=False)        # [i, t, e]
    xt_d = nc.declare_dram_parameter("xt", [P, CHUNK], BF16, isOutput=False)     # [d, p]
    halo_d = nc.declare_dram_parameter("halo", [P, 2, P], F32, isOutput=False)   # [i, u, e]
    w2_d = nc.declare_dram_parameter("w2", [P, P * P], BF16, isOutput=False)     # [d, (e,f)]
    lt_d = nc.declare_dram_parameter("lt", [P, P], F32, isOutput=False)
    pow_d = nc.declare_dram_parameter("powv", [1, P], F32, isOutput=False)
    vw_d = nc.declare_dram_parameter("vw", [P, 1], F32, isOutput=False)
    hw_d = nc.declare_dram_parameter("hw", [P, 2], F32, isOutput=False)
    lt9_d = nc.declare_dram_parameter("lt9", [NT + 1, NT], F32, isOutput=False)
    out_d = nc.declare_dram_parameter("out", [P, NT, P], F32, isOutput=True)  # [p, t, f]

    mult = mybir.AluOpType.mult
    add = mybir.AluOpType.add
    routes = _chunk_routes()

    with tile.TileContext(nc) as tc:
        with tc.tile_pool(name="consts", bufs=1) as consts:
            w2_sb = [consts.tile([P, 2048], BF16, name=f"w2_sb{i}")
                     for i in range(8)]
            xt_sb = consts.tile([P, CHUNK], BF16)
            x_sb = consts.tile([P, NT, P], F32)
            halo_sb = consts.tile([P, 2, P], F32)
            lt_sb = consts.tile([P, P], F32)
            pow_sb = consts.tile([1, P], F32)
            vw_sb = consts.tile([P, 1], F32)
            hw_sb = consts.tile([P, 2], F32)
            lt9_sb = consts.tile([NT + 1, NT], F32)
            v9_sb = consts.tile([NT + 1, P], F32)
            c0_sb = consts.tile([1, P], F32)
            va_sb = consts.tile([1, 4 * P], F32)
            vb_sb = consts.tile([1, 4 * P], F32)
            c8_sb = consts.tile([NT, P], F32)
            c_all = consts.tile([1, NT * P], F32)    # [1, (t,e)] carries
            s_sb = consts.tile([P, NT, P], F32)      # [p, t, e]
            acc_v = consts.tile([P, NT, P], F32)     # [p, t, f] DVE accumulator
            acc_g = consts.tile([P, NT, P], F32)     # [p, t, f] GPSIMD accumulator
            acc = consts.tile([P, NT, P], F32)       # final sum

            for i in range(8):
                nc.sync.dma_start(out=w2_sb[i][:, :],
                                  in_=w2_d[:, ds(2048 * i, 2048)])
            nc.sync.dma_start(out=xt_sb[:, :], in_=xt_d[:, :])
            nc.sync.dma_start(out=x_sb[:, :, :], in_=x_d[:, :, :])
            nc.sync.dma_start(out=halo_sb[:, :, :], in_=halo_d[:, :, :])
            nc.sync.dma_start(out=lt_sb[:, :], in_=lt_d[:, :])
            nc.sync.dma_start(out=pow_sb[:, :], in_=pow_d[:, :])
            nc.sync.dma_start(out=vw_sb[:, :], in_=vw_d[:, :])
            nc.sync.dma_start(out=hw_sb[:, :], in_=hw_d[:, :])
            nc.sync.dma_start(out=lt9_sb[:, :], in_=lt9_d[:, :])

            nc.vector.memset(acc_v[:, :, :], 0.0)
            nc.gpsimd.memset(acc_g[:, :, :], 0.0)

            # ---- carries: c_t = s[tile_start t] for all 8 tiles ----
            with tc.tile_pool(name="psum_c", bufs=1, space="PSUM") as psum_c:
                c0_ps = psum_c.tile([1, P], F32)
                nc.tensor.matmul(c0_ps[:, :], lhsT=hw_sb[:, 0:1],
                                 rhs=halo_sb[:, 0, :], start=True, stop=False)
                nc.tensor.matmul(c0_ps[:, :], lhsT=hw_sb[:, 1:2],
                                 rhs=halo_sb[:, 1, :], start=False, stop=True)
                vps_a = psum_c.tile([1, 4 * P], F32, tag="vps_a")
                vps_b = psum_c.tile([1, 4 * P], F32, tag="vps_b")
                nc.tensor.matmul(vps_a[:, :], lhsT=vw_sb[:, :],
                                 rhs=x_sb[:, 0:4, :], start=True, stop=True)
                nc.tensor.matmul(vps_b[:, :], lhsT=vw_sb[:, :],
                                 rhs=x_sb[:, 4:8, :], start=True, stop=True)
                nc.vector.tensor_copy(c0_sb[:, :], c0_ps[:, :])
                nc.vector.tensor_copy(va_sb[:, :], vps_a[:, :])
                nc.vector.tensor_copy(vb_sb[:, :], vps_b[:, :])
                nc.sync.dma_start(out=v9_sb[0:1, :], in_=c0_sb[:, :])
                nc.sync.dma_start(out=v9_sb[1:5, :], in_=va_sb[:, :])
                nc.sync.dma_start(out=v9_sb[5:9, :], in_=vb_sb[:, :])
                c_ps = psum_c.tile([NT, P], F32, tag="c_ps")
                nc.tensor.matmul(c_ps[:, :], lhsT=lt9_sb[:, :],
                                 rhs=v9_sb[:, :], start=True, stop=True)
                nc.vector.tensor_copy(c8_sb[:, :], c_ps[:, :])
                nc.sync.dma_start(out=c_all[:, :], in_=c8_sb[:, :])

            # ---- s tiles: s = L @ x_t + pow (x) c_t ----
            with tc.tile_pool(name="psum_s", bufs=2, space="PSUM") as psum_s:
                for t in range(NT):
                    sp = psum_s.tile([P, P], F32)
                    nc.tensor.matmul(sp[:, :], lhsT=lt_sb[:, :],
                                     rhs=x_sb[:, t, :], start=True, stop=False)
                    nc.tensor.matmul(sp[:, :], lhsT=pow_sb[:, :],
                                     rhs=c_all[:, ts(t, P)], start=False, stop=True)
                    nc.vector.tensor_copy(s_sb[:, t, :], sp[:, :])

            # ---- main: Y = xT_t.T @ W2 chunks (bf16); 3-path contraction ----
            with tc.tile_pool(name="psum_y", bufs=6, space="PSUM") as psum_y, \
                 tc.tile_pool(name="yc_ring", bufs=12) as yc_ring, \
                 tc.tile_pool(name="pe_ring", bufs=24) as pe_ring:
                for t in range(NT):
                    xt_t = xt_sb[:, ts(t, P)]
                    for c in range(32):
                        yp = psum_y.tile([P, 512], F32)
                        nc.tensor.matmul(
                            yp[:, :], lhsT=xt_t,
                            rhs=w2_sb[c // 4][:, ds(512 * (c % 4), 512)],
                            start=True, stop=True)
                        route = routes[t][c]
                        if route == "direct":
                            for jj in range(4):
                                e = 4 * c + jj
                                nc.vector.scalar_tensor_tensor(
                                    out=acc_v[:, t, :],
                                    in0=yp[:, ts(jj, P)],
                                    scalar=s_sb[:, t, e:e + 1],
                                    in1=acc_v[:, t, :],
                                    op0=mult, op1=add)
                        elif route == "copied":
                            yc = yc_ring.tile([P, 512], BF16)
                            nc.scalar.copy(yc[:, :], yp[:, :])
                            for jj in range(4):
                                e = 4 * c + jj
                                nc.vector.scalar_tensor_tensor(
                                    out=acc_v[:, t, :],
                                    in0=yc[:, ts(jj, P)],
                                    scalar=s_sb[:, t, e:e + 1],
                                    in1=acc_v[:, t, :],
                                    op0=mult, op1=add)
                        else:  # gp: ACT per-e scaled copies + GPSIMD adds
                            for jj in range(4):
                                e = 4 * c + jj
                                pe_t = pe_ring.tile([P, P], BF16)
                                nc.scalar.activation(
                                    pe_t[:, :], yp[:, ts(jj, P)],
                                    mybir.ActivationFunctionType.Copy,
                                    scale=s_sb[:, t, e:e + 1])
                                nc.gpsimd.tensor_tensor(
                                    out=acc_g[:, t, :],
                                    in0=pe_t[:, :],
                                    in1=acc_g[:, t, :],
                                    op=add)

            # ---- combine accumulators and store ----
            nc.vector.tensor_tensor(
                out=acc[:, :, :], in0=acc_v[:, :, :], in1=acc_g[:, :, :],
                op=add)
            nc.sync.dma_start(out=out_d[:, :, :], in_=acc[:, :, :])
    nc.finalize()
    return nc


def _get_nc():
    if "nc" not in _CACHE:
        _CACHE["nc"] = _build_nc()
    return _CACHE["nc"]


def kernel(x, concept_map, _trace=False):
    global LAST_RESULTS
    x = np.asarray(x, dtype=np.float32)
    cm = np.asarray(concept_map, dtype=np.float32)
    assert x.shape == (B, S, D) and cm.shape == (D, D, D)

    consts = _host_constants()
    # W2[d, e*128+f] = cm[f, d, e]
    w2 = np.ascontiguousarray(
        np.transpose(cm, (1, 2, 0)).reshape(D, D * D)).astype(ml_dtypes.bfloat16)

    in_maps = []
    for core in range(NCORES):
        b, half = divmod(core, 2)
        lo = half * CHUNK
        xc = x[b, lo:lo + CHUNK]                          # [1024, 128]
        # [i, t, e] interleaved layout (partition = within-tile position)
        x_il = np.ascontiguousarray(
            xc.reshape(NT, P, D).transpose(1, 0, 2))
        xt = np.ascontiguousarray(xc.T).astype(ml_dtypes.bfloat16)  # [d, p]
        if half == 0:
            halo = np.zeros((P, 2, D), dtype=np.float32)
        else:
            h = x[b, lo - HALO:lo]                        # [256, 128]
            halo = np.ascontiguousarray(h.reshape(2, P, D).transpose(1, 0, 2))
        in_maps.append({
            "x": x_il, "xt": xt, "halo": halo, "w2": w2, **consts,
        })

    nc = _get_nc()
    res = run_bass_kernel_spmd(nc, in_maps, list(range(NCORES)), trace=_trace)
    LAST_RESULTS = res

    out = np.empty((B, S, D), dtype=np.float32)
    for core in range(NCORES):
        b, half = divmod(core, 2)
        o = res.results[core]["out"]                      # [p, t, f]
        out[b, half * CHUNK:(half + 1) * CHUNK] = (
            o.transpose(1, 0, 2).reshape(CHUNK, D))
    return out


# revision 6
# speedup vs baseline: 1.5663x; 1.2102x over previous
"""Trainium2 Bass kernel for nn_Head_75118978007668.

Computes, for x:[B,S,D], concept_map(cm):[D,D,D] (B=4, S=2048, D=128):
    s[b,t] = sum_{j<t} lam^(t-j) x[b,j]          (lam = 1/1.2 decayed prefix sum)
    out[b,t,f] = sum_{d,e} x[b,t,d] * s[b,t,e] * cm[f,d,e]

Sharding: 8 cores, each owns 1024 contiguous positions of one batch row
(4 rows x 2 halves).  The scan carry across the half-split is recovered
exactly (to fp32) from a 256-position halo (lam^256 ~ 4.5e-21 << fp32 eps).

Per-core dataflow (positions tiled 8 x 128):
  - carries + s tiles: small PE matmuls (as before), s stored [p,t,e,1] fp32
  - main per tile: Y = xT_t.T @ W2 in N=512 matmuls (bf16), grouped as
      * 11 "pair" groups (8 e's = 2 matmuls -> one 2-bank PSUM tile):
        DVE 1024-wide broadcast mult  Z = Y * s[.,e-range]  (stride-0 AP),
        GPSIMD 1024-wide bf16 add onto an 8-lane accumulator
      * 10 "c" chunks (4 e's = 1 matmul -> 1-bank PSUM tile):
        4x ACT scaled copies (scale=s) -> bf16, DVE 512-wide add onto a
        4-lane accumulator
  - lane-fold per tile (DVE bf16 tree) -> acc[p,t,f] fp32 -> DMA out
  where W2[d, e*128+f] = cm[f, d, e]  (host-transposed).
"""

import numpy as np
import ml_dtypes

import concourse.bass as bass
import concourse.tile as tile
from concourse import bacc, mybir
from concourse.bass import ds, ts
from concourse.bass_utils import run_bass_kernel_spmd

B, S, D = 4, 2048, 128
NCORES = 8
CHUNK = S // 2          # positions per core (1024)
NT = CHUNK // 128       # position tiles per core (8)
P = 128
HALO = 256
F32 = mybir.dt.float32
BF16 = mybir.dt.bfloat16

# match the reference's fp32 constant 1.2 exactly
LAM = 1.0 / np.float64(np.float32(1.2))

# per tile: NPAIR pair-groups of 8 e's + NC chunks of 4 e's = 128 e's
NPAIR = 11
NC = 10
assert NPAIR * 8 + NC * 4 == 128
# of the NPAIR pair-groups, this many get DVE adds instead of GPSIMD
NPAIR_DVE = 0

_CACHE = {}
LAST_RESULTS = None


def _host_constants():
    k = np.arange(P, dtype=np.float64)
    i = k
    LT = np.where(i[:, None] < k[None, :], LAM ** (k[None, :] - i[:, None]), 0.0)
    powv = (LAM ** k)[None, :]                      # [1, 128]
    vw = (LAM ** (P - i))[:, None]                  # [128, 1]
    j = np.arange(HALO, dtype=np.float64)
    hw = (LAM ** (HALO - j)).reshape(2, P).T        # [128, 2]
    t = np.arange(NT, dtype=np.float64)
    M9 = np.zeros((NT, NT + 1), dtype=np.float64)
    M9[:, 0] = LAM ** (P * t)
    for tt in range(NT):
        for jj in range(tt):
            M9[tt, jj + 1] = LAM ** (P * (tt - 1 - jj))
    LT9 = M9.T                                      # [9, 8]
    f32 = np.float32
    return {
        "lt": LT.astype(f32),
        "powv": powv.astype(f32),
        "vw": vw.astype(f32),
        "hw": hw.astype(f32),
        "lt9": LT9.astype(f32),
    }


def _emit_order():
    """Interleave NC c-chunks among NPAIR pair-groups for pipelining."""
    order = []
    ic, ip = 0, 0
    acc = 0.0
    for _ in range(NPAIR + NC):
        acc += NC / (NPAIR + NC)
        if acc >= 1.0 and ic < NC:
            order.append(("c", ic)); ic += 1; acc -= 1.0
        elif ip < NPAIR:
            order.append(("p", ip)); ip += 1
        else:
            order.append(("c", ic)); ic += 1
    return order


def _build_nc():
    nc = bacc.Bacc("TRN2", target_bir_lowering=False, debug=False,
                   num_devices=NCORES)
    x_d = nc.declare_dram_parameter("x", [P, NT, P], F32, isOutput=False)
    xt_d = nc.declare_dram_parameter("xt", [P, CHUNK], BF16, isOutput=False)
    halo_d = nc.declare_dram_parameter("halo", [P, 2, P], F32, isOutput=False)
    w2_d = nc.declare_dram_parameter("w2", [P, P * P], BF16, isOutput=False)
    lt_d = nc.declare_dram_parameter("lt", [P, P], F32, isOutput=False)
    pow_d = nc.declare_dram_parameter("powv", [1, P], F32, isOutput=False)
    vw_d = nc.declare_dram_parameter("vw", [P, 1], F32, isOutput=False)
    hw_d = nc.declare_dram_parameter("hw", [P, 2], F32, isOutput=False)
    lt9_d = nc.declare_dram_parameter("lt9", [NT + 1, NT], F32, isOutput=False)
    out_d = nc.declare_dram_parameter("out", [P, NT, P], F32, isOutput=True)

    mult = mybir.AluOpType.mult
    add = mybir.AluOpType.add
    order = _emit_order()

    with tile.TileContext(nc) as tc:
        with tc.tile_pool(name="consts", bufs=1) as consts:
            w2_sb = [consts.tile([P, 2048], BF16, name=f"w2_sb{i}")
                     for i in range(8)]
            xt_sb = consts.tile([P, CHUNK], BF16)
            x_sb = consts.tile([P, NT, P], F32)
            halo_sb = consts.tile([P, 2, P], F32)
            lt_sb = consts.tile([P, P], F32)
            pow_sb = consts.tile([1, P], F32)
            vw_sb = consts.tile([P, 1], F32)
            hw_sb = consts.tile([P, 2], F32)
            lt9_sb = consts.tile([NT + 1, NT], F32)
            v9_sb = consts.tile([NT + 1, P], F32)
            c0_sb = consts.tile([1, P], F32)
            va_sb = consts.tile([1, 4 * P], F32)
            vb_sb = consts.tile([1, 4 * P], F32)
            c8_sb = consts.tile([NT, P], F32)
            c_all = consts.tile([1, NT * P], F32)
            s_sb = consts.tile([P, NT, P, 1], F32)   # [p, t, e, 1]
            acc = consts.tile([P, NT, P], F32)       # [p, t, f] final

            for i in range(8):
                nc.sync.dma_start(out=w2_sb[i][:, :],
                                  in_=w2_d[:, ds(2048 * i, 2048)])
            nc.sync.dma_start(out=xt_sb[:, :], in_=xt_d[:, :])
            nc.sync.dma_start(out=x_sb[:, :, :], in_=x_d[:, :, :])
            nc.sync.dma_start(out=halo_sb[:, :, :], in_=halo_d[:, :, :])
            nc.sync.dma_start(out=lt_sb[:, :], in_=lt_d[:, :])
            nc.sync.dma_start(out=pow_sb[:, :], in_=pow_d[:, :])
            nc.sync.dma_start(out=vw_sb[:, :], in_=vw_d[:, :])
            nc.sync.dma_start(out=hw_sb[:, :], in_=hw_d[:, :])
            nc.sync.dma_start(out=lt9_sb[:, :], in_=lt9_d[:, :])

            # ---- carries ----
            with tc.tile_pool(name="psum_c", bufs=1, space="PSUM") as psum_c:
                c0_ps = psum_c.tile([1, P], F32)
                nc.tensor.matmul(c0_ps[:, :], lhsT=hw_sb[:, 0:1],
                                 rhs=halo_sb[:, 0, :], start=True, stop=False)
                nc.tensor.matmul(c0_ps[:, :], lhsT=hw_sb[:, 1:2],
                                 rhs=halo_sb[:, 1, :], start=False, stop=True)
                vps_a = psum_c.tile([1, 4 * P], F32, tag="vps_a")
                vps_b = psum_c.tile([1, 4 * P], F32, tag="vps_b")
                nc.tensor.matmul(vps_a[:, :], lhsT=vw_sb[:, :],
                                 rhs=x_sb[:, 0:4, :], start=True, stop=True)
                nc.tensor.matmul(vps_b[:, :], lhsT=vw_sb[:, :],
                                 rhs=x_sb[:, 4:8, :], start=True, stop=True)
                nc.vector.tensor_copy(c0_sb[:, :], c0_ps[:, :])
                nc.vector.tensor_copy(va_sb[:, :], vps_a[:, :])
                nc.vector.tensor_copy(vb_sb[:, :], vps_b[:, :])
                nc.sync.dma_start(out=v9_sb[0:1, :], in_=c0_sb[:, :])
                nc.sync.dma_start(out=v9_sb[1:5, :], in_=va_sb[:, :])
                nc.sync.dma_start(out=v9_sb[5:9, :], in_=vb_sb[:, :])
                c_ps = psum_c.tile([NT, P], F32, tag="c_ps")
                nc.tensor.matmul(c_ps[:, :], lhsT=lt9_sb[:, :],
                                 rhs=v9_sb[:, :], start=True, stop=True)
                nc.vector.tensor_copy(c8_sb[:, :], c_ps[:, :])
                nc.sync.dma_start(out=c_all[:, :], in_=c8_sb[:, :])

            # ---- s tiles: s = L @ x_t + pow (x) c_t  -> [p,t,e,1] ----
            with tc.tile_pool(name="psum_s", bufs=2, space="PSUM") as psum_s:
                for t in range(NT):
                    sp = psum_s.tile([P, P, 1], F32)
                    nc.tensor.matmul(sp[:, :, :], lhsT=lt_sb[:, :],
                                     rhs=x_sb[:, t, :], start=True, stop=False)
                    nc.tensor.matmul(sp[:, :, :], lhsT=pow_sb[:, :],
                                     rhs=c_all[:, ts(t, P)], start=False, stop=True)
                    nc.vector.tensor_copy(s_sb[:, t, :, :], sp[:, :, :])

            # ---- main ----
            with tc.tile_pool(name="psum_p", bufs=2, space="PSUM") as psum_p, \
                 tc.tile_pool(name="psum_cc", bufs=4, space="PSUM") as psum_cc, \
                 tc.tile_pool(name="zp_ring", bufs=4) as zp_ring, \
                 tc.tile_pool(name="zc_ring", bufs=4) as zc_ring, \
                 tc.tile_pool(name="accp", bufs=2) as accp:
                for t in range(NT):
                    xt_t = xt_sb[:, ts(t, P)]
                    acc8g = accp.tile([P, 8, P], BF16, name="acc8g", tag="g")
                    acc4v = accp.tile([P, 4, P], BF16, name="acc4v", tag="v")
                    nc.gpsimd.memset(acc8g[:, :, :], 0.0)
                    nc.vector.memset(acc4v[:, :, :], 0.0)

                    for kind, idx in order:
                        if kind == "p":
                            e0 = 8 * idx                     # e range [e0, e0+8)
                            yq = psum_p.tile([P, 8, P], F32)
                            for h in range(2):
                                cset = (e0 + 4 * h) // 4     # 512-col chunk idx
                                nc.tensor.matmul(
                                    yq[:, ds(4 * h, 4), :], lhsT=xt_t,
                                    rhs=w2_sb[cset // 4][:, ds(512 * (cset % 4), 512)],
                                    start=True, stop=True)
                            zp = zp_ring.tile([P, 8, P], BF16)
                            sb_b = s_sb[:, t, ds(e0, 8), :].broadcast_to([P, 8, P])
                            nc.vector.tensor_tensor(
                                out=zp[:, :, :], in0=yq[:, :, :], in1=sb_b,
                                op=mult)
                            nc.gpsimd.tensor_tensor(
                                out=acc8g[:, :, :], in0=zp[:, :, :],
                                in1=acc8g[:, :, :], op=add)
                        else:
                            e0 = 8 * NPAIR + 4 * idx
                            cset = e0 // 4
                            yc = psum_cc.tile([P, 4, P], F32)
                            nc.tensor.matmul(
                                yc[:, :, :], lhsT=xt_t,
                                rhs=w2_sb[cset // 4][:, ds(512 * (cset % 4), 512)],
                                start=True, stop=True)
                            zc = zc_ring.tile([P, 4, P], BF16)
                            for jj in range(4):
                                e = e0 + jj
                                nc.scalar.activation(
                                    zc[:, jj, :], yc[:, jj, :],
                                    mybir.ActivationFunctionType.Copy,
                                    scale=s_sb[:, t, e, :])
                            nc.vector.tensor_tensor(
                                out=acc4v[:, :, :], in0=zc[:, :, :],
                                in1=acc4v[:, :, :], op=add)

                    # ---- lane fold: (8 lanes g) + (4 lanes v) -> acc[p,t,:] ----
                    f4 = accp.tile([P, 4, P], BF16, name="f4", tag="f4")
                    nc.vector.tensor_tensor(
                        out=f4[:, :, :], in0=acc8g[:, 0:4, :],
                        in1=acc8g[:, 4:8, :], op=add)
                    f4b = accp.tile([P, 4, P], BF16, name="f4b", tag="f4b")
                    nc.vector.tensor_tensor(
                        out=f4b[:, :, :], in0=f4[:, :, :], in1=acc4v[:, :, :],
                        op=add)
                    f2 = accp.tile([P, 2, P], BF16, name="f2", tag="f2")
                    nc.vector.tensor_tensor(
                        out=f2[:, :, :], in0=f4b[:, 0:2, :], in1=f4b[:, 2:4, :],
                        op=add)
                    nc.vector.tensor_tensor(
                        out=acc[:, t, :], in0=f2[:, 0, :], in1=f2[:, 1, :],
                        op=add)

            nc.sync.dma_start(out=out_d[:, :, :], in_=acc[:, :, :])
    nc.finalize()
    return nc


def _get_nc():
    if "nc" not in _CACHE:
        _CACHE["nc"] = _build_nc()
    return _CACHE["nc"]


def kernel(x, concept_map, _trace=False):
    global LAST_RESULTS
    x = np.asarray(x, dtype=np.float32)
    cm = np.asarray(concept_map, dtype=np.float32)
    assert x.shape == (B, S, D) and cm.shape == (D, D, D)

    consts = _host_constants()
    w2 = np.ascontiguousarray(
        np.transpose(cm, (1, 2, 0)).reshape(D, D * D)).astype(ml_dtypes.bfloat16)

    in_maps = []
    for core in range(NCORES):
        b, half = divmod(core, 2)
        lo = half * CHUNK
        xc = x[b, lo:lo + CHUNK]
        x_il = np.ascontiguousarray(
            xc.reshape(NT, P, D).transpose(1, 0, 2))
        xt = np.ascontiguousarray(xc.T).astype(ml_dtypes.bfloat16)
        if half == 0:
            halo = np.zeros((P, 2, D), dtype=np.float32)
        else:
            h = x[b, lo - HALO:lo]
            halo = np.ascontiguousarray(h.reshape(2, P, D).transpose(1, 0, 2))
        in_maps.append({
            "x": x_il, "xt": xt, "halo": halo, "w2": w2, **consts,
        })

    nc = _get_nc()
    res = run_bass_kernel_spmd(nc, in_maps, list(range(NCORES)), trace=_trace)
    LAST_RESULTS = res

    out = np.empty((B, S, D), dtype=np.float32)
    for core in range(NCORES):
        b, half = divmod(core, 2)
        o = res.results[core]["out"]
        out[b, half * CHUNK:(half + 1) * CHUNK] = (
            o.transpose(1, 0, 2).reshape(CHUNK, D))
    return out
